# revision 6
# baseline (speedup 1.0000x reference)
"""Trainium2 Bass kernel for nn_AEDecoder (sparse 2-layer gene decoder).

Computation (reference):
    h   = leaky_relu(features @ W1s.T + b1, 0.01)   # W1s sparse [80000, 1600], 10 nnz/row
    out = h @ W2s.T + b2                             # gene g sums its 4 hidden nodes

Structure exploited (deterministic from the reference's connectivity builder):
  - hidden node h = 4g+k uses the *same* 10 TFs for all k of gene g
  - layer 2 is a grouped reduction over contiguous blocks of 4

Strategy: shard genes across the 8 cores (2500/core, full batch per core; no
cross-core comms). Per core: dma_gather rows of transposed bf16 features
(batch on the free axis, 256 B per gathered row), then per 12-gene block one
bf16 matmul [128 slots, 64]^T @ [128 slots, 128 batch] -> PSUM, one ScalarE
Lrelu pass per 8-block htile, then one layer-2 matmul per 128-batch slot
(2 blocks fused via zero-padded rows), biases folded into the matmuls via a
constant ones row gathered from a 1601st feature row.
"""

import sys

sys.path.insert(0, "/opt/trn_rl_repo")

import numpy as np
import ml_dtypes

# ---- problem constants (hardcoded; kernel.py must be self-contained) ----
NUM_TFS = 1600
NUM_GENES = 20000
WIDTH = 4
TPG = 10  # TFs per gene
HIDDEN = NUM_GENES * WIDTH
BATCH = 128
NCORES = 8
GENES_PER_CORE = NUM_GENES // NCORES  # 2500

# ---- block geometry ----
GPB = 12  # genes per block (120 slots + 8 pad = 128 partitions)
BPH = 8  # blocks per htile: 2 col groups (u) x 4 free slots (v)
GENES_PER_HTILE = GPB * BPH  # 96
HTILES = -(-GENES_PER_CORE // GENES_PER_HTILE)  # 27
GENES_PAD = HTILES * GENES_PER_HTILE  # 2592
BLOCKS = HTILES * BPH  # 216
NIDX = BLOCKS * 128  # 27648 gather slots per core
GATHER_HTILES = 1  # htiles per dma_gather (HW limit: <=1024 idxs per gather)
ONES_ROW = NUM_TFS  # feature row index holding constant 1.0

_PROGRAM = None


def _build_program(htiles=HTILES):
    import concourse.bacc as bacc
    import concourse.mybir as mybir
    from concourse import tile
    from concourse.library_config import mlp

    dt = mybir.dt
    AF = mybir.ActivationFunctionType

    nc = bacc.Bacc("TRN2")
    f = nc.dram_tensor("f", [NUM_TFS + 1, BATCH], dt.bfloat16, kind="ExternalInput")
    idx = nc.dram_tensor("idx", [128, NIDX // 16], dt.int16, kind="ExternalInput")
    w1 = nc.dram_tensor(
        "w1b", [HTILES, 128, BPH * 64], dt.bfloat16, kind="ExternalInput"
    )
    w2 = nc.dram_tensor("w2b", [HTILES, 128, 4 * 32], dt.bfloat16, kind="ExternalInput")
    out = nc.dram_tensor("out", [GENES_PAD, BATCH], dt.float32, kind="ExternalOutput")

    with tile.TileContext(nc) as tc:
        nc.gpsimd.load_library(mlp)
        with (
            tc.tile_pool(name="idxp", bufs=1) as idxpool,
            tc.tile_pool(name="g", bufs=3) as gpool,
            tc.tile_pool(name="w1p", bufs=3) as w1pool,
            tc.tile_pool(name="w2p", bufs=3) as w2pool,
            tc.tile_pool(name="h", bufs=3) as hpool,
            tc.tile_pool(name="o", bufs=2) as opool,
            tc.tile_pool(name="hps", bufs=3, space="PSUM") as hpsum,
            tc.tile_pool(name="ops", bufs=2, space="PSUM") as opsum,
        ):
            idxt = idxpool.tile([128, NIDX // 16], dt.int16)
            nc.sync.dma_start(idxt[:], idx[:])

            gt = None
            otile = None
            for t in range(htiles):
                if t % GATHER_HTILES == 0:
                    nh = min(GATHER_HTILES, htiles - t)
                    n_i = nh * BPH * 128
                    gt = gpool.tile([128, nh * BPH, 128], dt.bfloat16, tag="g")
                    c0 = t * BPH * 128 // 16
                    nc.gpsimd.dma_gather(
                        gt[:],
                        f[:],
                        idxt[:, c0 : c0 + n_i // 16],
                        n_i,
                        n_i,
                        BATCH,
                    )
                w1t = w1pool.tile([128, BPH * 64], dt.bfloat16)
                nc.sync.dma_start(w1t[:], w1[t])
                w2t = w2pool.tile([128, 4 * 32], dt.bfloat16)
                nc.sync.dma_start(w2t[:], w2[t])

                hps = hpsum.tile([128, 512], dt.float32)
                for ci in range(BPH):
                    u, v = ci // 4, ci % 4
                    nc.tensor.matmul(
                        hps[64 * u : 64 * u + 64, 128 * v : 128 * v + 128],
                        w1t[:, 64 * ci : 64 * ci + 64],
                        gt[:, (t % GATHER_HTILES) * BPH + ci, :],
                        start=True,
                        stop=True,
                    )
                hsb = hpool.tile([128, 512], dt.bfloat16)
                nc.scalar.activation(
                    hsb[:], hps[:], AF.Prelu, bias=0.0, scale=1.0, alpha=0.01
                )

                j = t % 3
                if j == 0:
                    otile = opsum.tile([96, 512], dt.float32, tag="o")
                for v in range(4):
                    nc.tensor.matmul(
                        otile[32 * j : 32 * j + 32, 128 * v : 128 * v + 128],
                        w2t[:, 32 * v : 32 * v + 32],
                        hsb[:, 128 * v : 128 * v + 128],
                        start=True,
                        stop=True,
                    )
                if j == 2:
                    osb = opool.tile([96, 512], dt.float32)
                    nc.vector.tensor_copy(osb[:], otile[:])
                    for jj in range(3):
                        th = t - 2 + jj
                        src = osb[32 * jj : 32 * jj + 24, :].rearrange(
                            "r (v b) -> r v b", v=4
                        )
                        dst = out[
                            GENES_PER_HTILE * th : GENES_PER_HTILE * (th + 1), :
                        ].rearrange("(v r) b -> r v b", v=4)
                        nc.sync.dma_start(dst, src)
    nc.compile()
    return nc


def _get_program():
    global _PROGRAM
    if _PROGRAM is None:
        _PROGRAM = _build_program()
    return _PROGRAM


def _prep_inputs(features, w1, b1, w2, b2, rows1, cols1, rows2, cols2):
    """Host-side staging: shard genes over cores, densify per-block weights.

    Returns in_maps: list (per core) of dicts with keys f/idx/w1b/w2b.
    """
    features = np.asarray(features, dtype=np.float32)
    w1 = np.asarray(w1, dtype=np.float32)
    b1 = np.asarray(b1, dtype=np.float32)
    w2 = np.asarray(w2, dtype=np.float32)
    b2 = np.asarray(b2, dtype=np.float32)
    cols1 = np.asarray(cols1)

    # Verify the structural assumption this kernel exploits.
    c1 = cols1.reshape(NUM_GENES, WIDTH, TPG)
    assert (c1 == c1[:, :1, :]).all(), "layer-1 TFs must be shared across WIDTH"
    tf_idx = c1[:, 0, :].astype(np.int64)  # [G, 10]

    w1g = w1.reshape(NUM_GENES, WIDTH, TPG)  # [G, 4, 10]
    b1g = b1.reshape(NUM_GENES, WIDTH)  # [G, 4]
    w2g = w2.reshape(NUM_GENES, WIDTH)  # [G, 4]

    # features.T with an appended ones row (bias carrier), bf16.
    F = np.empty((NUM_TFS + 1, BATCH), dtype=ml_dtypes.bfloat16)
    F[:NUM_TFS] = features.T.astype(ml_dtypes.bfloat16)
    F[NUM_TFS] = 1.0

    in_maps = []
    for c in range(NCORES):
        w1b = np.zeros((HTILES, 128, BPH * 64), dtype=np.float32)
        w2b = np.zeros((HTILES, 128, 4 * 32), dtype=np.float32)
        idxs = np.full((HTILES, BPH, 128), ONES_ROW, dtype=np.int64)
        for t in range(HTILES):
            for ci in range(BPH):
                u, v = ci // 4, ci % 4
                gs = 96 * t + 24 * v + 12 * u + np.arange(GPB)
                val = gs < GENES_PER_CORE
                g = np.where(val, c * GENES_PER_CORE + gs, 0)
                wv = w1g[g] * val[:, None, None]  # [12, 4, 10]
                for j in range(GPB):
                    # layer-1 weights: row 10j+tt, col 64ci + 4j+k
                    w1b[t, 10 * j : 10 * j + 10, 64 * ci + 4 * j : 64 * ci + 4 * j + 4] = (
                        wv[j].T
                    )
                    # b1 via the ones gather slot (row 120)
                    w1b[t, 120, 64 * ci + 4 * j : 64 * ci + 4 * j + 4] = (
                        b1g[g[j]] * val[j]
                    )
                    # layer-2 weights: row 64u+4j+k, col 24v + 12u + j
                    w2b[t, 64 * u + 4 * j : 64 * u + 4 * j + 4, 32 * v + 12 * u + j] = (
                        w2g[g[j]] * val[j]
                    )
                    # b2 via the ones h-row (48)
                    w2b[t, 48, 32 * v + 12 * u + j] = b2[g[j]] * val[j]
                if u == 0:
                    # h row 48 := 1.0 (written by the u=0 matmul's col 48)
                    w1b[t, 120, 64 * ci + 48] = 1.0
                idxs[t, ci, :120] = np.where(
                    val[:, None], tf_idx[g], ONES_ROW
                ).reshape(120)
        flat = idxs.reshape(NIDX).astype(np.int16)
        wrapped = np.tile(flat.reshape(NIDX // 16, 16).T, (8, 1))  # [128, NIDX/16]

        in_maps.append(
            {
                "f": F,
                "idx": np.ascontiguousarray(wrapped),
                "w1b": w1b.astype(ml_dtypes.bfloat16),
                "w2b": w2b.astype(ml_dtypes.bfloat16),
            }
        )
    return in_maps


def _assemble_output(core_outs):
    parts = [o[:GENES_PER_CORE] for o in core_outs]
    full = np.concatenate(parts, axis=0)  # [20000, 128]
    return np.ascontiguousarray(full.T.astype(np.float32))


def kernel(**inputs) -> np.ndarray:
    from concourse.bass_utils import run_bass_kernel_spmd

    nc = _get_program()
    in_maps = _prep_inputs(**inputs)
    res = run_bass_kernel_spmd(nc, in_maps, core_ids=list(range(NCORES)))
    return _assemble_output([r["out"] for r in res.results])


# revision 7
# speedup vs baseline: 1.7291x; 1.7291x over previous
"""Trainium2 Bass kernel for nn_AEDecoder (sparse 2-layer gene decoder).

Computation (reference):
    h   = leaky_relu(features @ W1s.T + b1, 0.01)   # W1s sparse [80000, 1600], 10 nnz/row
    out = h @ W2s.T + b2                             # gene g sums its 4 hidden nodes

Structure exploited (deterministic from the reference's connectivity builder):
  - hidden node h = 4g+k uses the *same* 10 TFs for all k of gene g
  - layer 2 is a grouped reduction over contiguous blocks of 4

Strategy: shard genes across the 8 cores (2500/core, full batch per core; no
cross-core comms). Per core: dma_gather rows of transposed bf16 features
(batch on the free axis, 256 B per gathered row), then per 12-gene block one
bf16 matmul [128 slots, 64]^T @ [128 slots, 128 batch] -> PSUM, one ScalarE
Lrelu pass per 8-block htile, then one layer-2 matmul per 128-batch slot
(2 blocks fused via zero-padded rows), biases folded into the matmuls via a
constant ones row gathered from a 1601st feature row.
"""

import sys

sys.path.insert(0, "/opt/trn_rl_repo")

import numpy as np
import ml_dtypes

# ---- problem constants (hardcoded; kernel.py must be self-contained) ----
NUM_TFS = 1600
NUM_GENES = 20000
WIDTH = 4
TPG = 10  # TFs per gene
HIDDEN = NUM_GENES * WIDTH
BATCH = 128
NCORES = 8
GENES_PER_CORE = NUM_GENES // NCORES  # 2500

# ---- block geometry ----
GPB = 12  # genes per block (120 slots + 8 pad = 128 partitions)
BPH = 8  # blocks per htile: 2 col groups (u) x 4 free slots (v)
GENES_PER_HTILE = GPB * BPH  # 96
HTILES = -(-GENES_PER_CORE // GENES_PER_HTILE)  # 27
GENES_PAD = HTILES * GENES_PER_HTILE  # 2592
BLOCKS = HTILES * BPH  # 216
NIDX = BLOCKS * 128  # 27648 gather slots per core
GATHER_HTILES = 1  # htiles per dma_gather (HW limit: <=1024 idxs per gather)
ONES_ROW = NUM_TFS  # feature row index holding constant 1.0

_PROGRAM = None


def _build_program(htiles=HTILES):
    import concourse.bacc as bacc
    import concourse.mybir as mybir
    from concourse import tile
    from concourse.library_config import mlp

    dt = mybir.dt
    AF = mybir.ActivationFunctionType

    nc = bacc.Bacc("TRN2", num_swdge_queues=4)
    f = nc.dram_tensor("f", [NUM_TFS + 1, BATCH], dt.bfloat16, kind="ExternalInput")
    idx = nc.dram_tensor("idx", [128, NIDX // 16], dt.int16, kind="ExternalInput")
    w1 = nc.dram_tensor(
        "w1b", [HTILES, 128, BPH * 64], dt.bfloat16, kind="ExternalInput"
    )
    w2 = nc.dram_tensor("w2b", [HTILES, 128, 4 * 32], dt.bfloat16, kind="ExternalInput")
    out = nc.dram_tensor("out", [GENES_PAD, BATCH], dt.float32, kind="ExternalOutput")

    with tile.TileContext(nc) as tc:
        nc.gpsimd.load_library(mlp)
        with (
            tc.tile_pool(name="idxp", bufs=1) as idxpool,
            tc.tile_pool(name="g", bufs=3) as gpool,
            tc.tile_pool(name="w1p", bufs=3) as w1pool,
            tc.tile_pool(name="w2p", bufs=3) as w2pool,
            tc.tile_pool(name="h", bufs=3) as hpool,
            tc.tile_pool(name="o", bufs=2) as opool,
            tc.tile_pool(name="hps", bufs=3, space="PSUM") as hpsum,
            tc.tile_pool(name="ops", bufs=2, space="PSUM") as opsum,
        ):
            idxt = idxpool.tile([128, NIDX // 16], dt.int16)
            nc.sync.dma_start(idxt[:], idx[:])

            gt = None
            otile = None
            for t in range(htiles):
                if t % GATHER_HTILES == 0:
                    nh = min(GATHER_HTILES, htiles - t)
                    n_i = nh * BPH * 128
                    gt = gpool.tile([128, nh * BPH, 128], dt.bfloat16, tag="g")
                    c0 = t * BPH * 128 // 16
                    nc.gpsimd.dma_gather(
                        gt[:],
                        f[:],
                        idxt[:, c0 : c0 + n_i // 16],
                        n_i,
                        n_i,
                        BATCH,
                        queue_num=t % 4,
                    )
                w1t = w1pool.tile([128, BPH * 64], dt.bfloat16)
                nc.sync.dma_start(w1t[:], w1[t])
                w2t = w2pool.tile([128, 4 * 32], dt.bfloat16)
                nc.sync.dma_start(w2t[:], w2[t])

                hps = hpsum.tile([128, 512], dt.float32)
                for ci in range(BPH):
                    u, v = ci // 4, ci % 4
                    nc.tensor.matmul(
                        hps[64 * u : 64 * u + 64, 128 * v : 128 * v + 128],
                        w1t[:, 64 * ci : 64 * ci + 64],
                        gt[:, (t % GATHER_HTILES) * BPH + ci, :],
                        start=True,
                        stop=True,
                    )
                hsb = hpool.tile([128, 512], dt.bfloat16)
                nc.scalar.activation(
                    hsb[:], hps[:], AF.Prelu, bias=0.0, scale=1.0, alpha=0.01
                )

                j = t % 3
                if j == 0:
                    otile = opsum.tile([96, 512], dt.float32, tag="o")
                for v in range(4):
                    nc.tensor.matmul(
                        otile[32 * j : 32 * j + 32, 128 * v : 128 * v + 128],
                        w2t[:, 32 * v : 32 * v + 32],
                        hsb[:, 128 * v : 128 * v + 128],
                        start=True,
                        stop=True,
                    )
                if j == 2:
                    osb = opool.tile([96, 512], dt.float32)
                    nc.vector.tensor_copy(osb[:], otile[:])
                    for jj in range(3):
                        th = t - 2 + jj
                        src = osb[32 * jj : 32 * jj + 24, :].rearrange(
                            "r (v b) -> r v b", v=4
                        )
                        dst = out[
                            GENES_PER_HTILE * th : GENES_PER_HTILE * (th + 1), :
                        ].rearrange("(v r) b -> r v b", v=4)
                        nc.sync.dma_start(dst, src)
    nc.compile()
    return nc


def _get_program():
    global _PROGRAM
    if _PROGRAM is None:
        _PROGRAM = _build_program()
    return _PROGRAM


def _prep_inputs(features, w1, b1, w2, b2, rows1, cols1, rows2, cols2):
    """Host-side staging: shard genes over cores, densify per-block weights.

    Returns in_maps: list (per core) of dicts with keys f/idx/w1b/w2b.
    """
    features = np.asarray(features, dtype=np.float32)
    w1 = np.asarray(w1, dtype=np.float32)
    b1 = np.asarray(b1, dtype=np.float32)
    w2 = np.asarray(w2, dtype=np.float32)
    b2 = np.asarray(b2, dtype=np.float32)
    cols1 = np.asarray(cols1)

    # Verify the structural assumption this kernel exploits.
    c1 = cols1.reshape(NUM_GENES, WIDTH, TPG)
    assert (c1 == c1[:, :1, :]).all(), "layer-1 TFs must be shared across WIDTH"
    tf_idx = c1[:, 0, :].astype(np.int64)  # [G, 10]

    w1g = w1.reshape(NUM_GENES, WIDTH, TPG)  # [G, 4, 10]
    b1g = b1.reshape(NUM_GENES, WIDTH)  # [G, 4]
    w2g = w2.reshape(NUM_GENES, WIDTH)  # [G, 4]

    # features.T with an appended ones row (bias carrier), bf16.
    F = np.empty((NUM_TFS + 1, BATCH), dtype=ml_dtypes.bfloat16)
    F[:NUM_TFS] = features.T.astype(ml_dtypes.bfloat16)
    F[NUM_TFS] = 1.0

    in_maps = []
    for c in range(NCORES):
        w1b = np.zeros((HTILES, 128, BPH * 64), dtype=np.float32)
        w2b = np.zeros((HTILES, 128, 4 * 32), dtype=np.float32)
        idxs = np.full((HTILES, BPH, 128), ONES_ROW, dtype=np.int64)
        for t in range(HTILES):
            for ci in range(BPH):
                u, v = ci // 4, ci % 4
                gs = 96 * t + 24 * v + 12 * u + np.arange(GPB)
                val = gs < GENES_PER_CORE
                g = np.where(val, c * GENES_PER_CORE + gs, 0)
                wv = w1g[g] * val[:, None, None]  # [12, 4, 10]
                for j in range(GPB):
                    # layer-1 weights: row 10j+tt, col 64ci + 4j+k
                    w1b[t, 10 * j : 10 * j + 10, 64 * ci + 4 * j : 64 * ci + 4 * j + 4] = (
                        wv[j].T
                    )
                    # b1 via the ones gather slot (row 120)
                    w1b[t, 120, 64 * ci + 4 * j : 64 * ci + 4 * j + 4] = (
                        b1g[g[j]] * val[j]
                    )
                    # layer-2 weights: row 64u+4j+k, col 24v + 12u + j
                    w2b[t, 64 * u + 4 * j : 64 * u + 4 * j + 4, 32 * v + 12 * u + j] = (
                        w2g[g[j]] * val[j]
                    )
                    # b2 via the ones h-row (48)
                    w2b[t, 48, 32 * v + 12 * u + j] = b2[g[j]] * val[j]
                if u == 0:
                    # h row 48 := 1.0 (written by the u=0 matmul's col 48)
                    w1b[t, 120, 64 * ci + 48] = 1.0
                idxs[t, ci, :120] = np.where(
                    val[:, None], tf_idx[g], ONES_ROW
                ).reshape(120)
        flat = idxs.reshape(NIDX).astype(np.int16)
        wrapped = np.tile(flat.reshape(NIDX // 16, 16).T, (8, 1))  # [128, NIDX/16]

        in_maps.append(
            {
                "f": F,
                "idx": np.ascontiguousarray(wrapped),
                "w1b": w1b.astype(ml_dtypes.bfloat16),
                "w2b": w2b.astype(ml_dtypes.bfloat16),
            }
        )
    return in_maps


def _assemble_output(core_outs):
    parts = [o[:GENES_PER_CORE] for o in core_outs]
    full = np.concatenate(parts, axis=0)  # [20000, 128]
    return np.ascontiguousarray(full.T.astype(np.float32))


def kernel(**inputs) -> np.ndarray:
    from concourse.bass_utils import run_bass_kernel_spmd

    nc = _get_program()
    in_maps = _prep_inputs(**inputs)
    res = run_bass_kernel_spmd(nc, in_maps, core_ids=list(range(NCORES)))
    return _assemble_output([r["out"] for r in res.results])


# revision 8
# speedup vs baseline: 1.9521x; 1.1289x over previous
"""Trainium2 Bass kernel for nn_AEDecoder (sparse 2-layer gene decoder).

Computation (reference):
    h   = leaky_relu(features @ W1s.T + b1, 0.01)   # W1s sparse [80000, 1600], 10 nnz/row
    out = h @ W2s.T + b2                             # gene g sums its 4 hidden nodes

Structure exploited (deterministic from the reference's connectivity builder):
  - hidden node h = 4g+k uses the *same* 10 TFs for all k of gene g
  - layer 2 is a grouped reduction over contiguous blocks of 4

Strategy: shard genes across the 8 cores (2500/core, full batch per core; no
cross-core comms). Per core: dma_gather rows of transposed bf16 features
(batch on the free axis, 256 B per gathered row), then per 12-gene block one
bf16 matmul [128 slots, 64]^T @ [128 slots, 128 batch] -> PSUM, one ScalarE
Lrelu pass per 8-block htile, then one layer-2 matmul per 128-batch slot
(2 blocks fused via zero-padded rows), biases folded into the matmuls via a
constant ones row gathered from a 1601st feature row.
"""

import sys

sys.path.insert(0, "/opt/trn_rl_repo")

import numpy as np
import ml_dtypes

# ---- problem constants (hardcoded; kernel.py must be self-contained) ----
NUM_TFS = 1600
NUM_GENES = 20000
WIDTH = 4
TPG = 10  # TFs per gene
HIDDEN = NUM_GENES * WIDTH
BATCH = 128
NCORES = 8
GENES_PER_CORE = NUM_GENES // NCORES  # 2500

# ---- block geometry ----
GPB = 12  # genes per block (120 slots + 8 pad = 128 partitions)
BPH = 8  # blocks per htile: 2 col groups (u) x 4 free slots (v)
GENES_PER_HTILE = GPB * BPH  # 96
HTILES = -(-GENES_PER_CORE // GENES_PER_HTILE)  # 27
GENES_PAD = HTILES * GENES_PER_HTILE  # 2592
BLOCKS = HTILES * BPH  # 216
NIDX = BLOCKS * 128  # 27648 gather slots per core
GATHER_HTILES = 1  # htiles per dma_gather (HW limit: <=1024 idxs per gather)
ONES_ROW = NUM_TFS  # feature row index holding constant 1.0

_PROGRAM = None


def _build_program(htiles=HTILES):
    import concourse.bacc as bacc
    import concourse.mybir as mybir
    from concourse import tile
    from concourse.library_config import mlp

    dt = mybir.dt
    AF = mybir.ActivationFunctionType

    nc = bacc.Bacc("TRN2", num_swdge_queues=4)
    f = nc.dram_tensor("f", [NUM_TFS + 1, BATCH], dt.bfloat16, kind="ExternalInput")
    idx = nc.dram_tensor("idx", [128, NIDX // 16], dt.int16, kind="ExternalInput")
    w1 = nc.dram_tensor(
        "w1b", [HTILES, 128, BPH * 64], dt.bfloat16, kind="ExternalInput"
    )
    w2 = nc.dram_tensor("w2b", [HTILES, 128, 4 * 32], dt.bfloat16, kind="ExternalInput")
    out = nc.dram_tensor("out", [GENES_PAD, BATCH], dt.float32, kind="ExternalOutput")

    with tile.TileContext(nc) as tc:
        nc.gpsimd.load_library(mlp)
        with (
            tc.tile_pool(name="idxp", bufs=1) as idxpool,
            tc.tile_pool(name="g", bufs=6) as gpool,
            tc.tile_pool(name="w1p", bufs=4) as w1pool,
            tc.tile_pool(name="w2p", bufs=4) as w2pool,
            tc.tile_pool(name="h", bufs=4) as hpool,
            tc.tile_pool(name="o", bufs=2) as opool,
            tc.tile_pool(name="hps", bufs=3, space="PSUM") as hpsum,
            tc.tile_pool(name="ops", bufs=2, space="PSUM") as opsum,
        ):
            idxt = idxpool.tile([128, NIDX // 16], dt.int16)
            nc.sync.dma_start(idxt[:], idx[:])

            gt = None
            otile = None
            for t in range(htiles):
                if t % GATHER_HTILES == 0:
                    nh = min(GATHER_HTILES, htiles - t)
                    n_i = nh * BPH * 128
                    gt = gpool.tile([128, nh * BPH, 128], dt.bfloat16, tag="g")
                    c0 = t * BPH * 128 // 16
                    nc.gpsimd.dma_gather(
                        gt[:],
                        f[:],
                        idxt[:, c0 : c0 + n_i // 16],
                        n_i,
                        n_i,
                        BATCH,
                        queue_num=t % 4,
                    )
                w1t = w1pool.tile([128, BPH * 64], dt.bfloat16)
                nc.sync.dma_start(w1t[:], w1[t])
                w2t = w2pool.tile([128, 4 * 32], dt.bfloat16)
                nc.sync.dma_start(w2t[:], w2[t])

                hps = hpsum.tile([128, 512], dt.float32)
                for ci in range(BPH):
                    u, v = ci // 4, ci % 4
                    nc.tensor.matmul(
                        hps[64 * u : 64 * u + 64, 128 * v : 128 * v + 128],
                        w1t[:, 64 * ci : 64 * ci + 64],
                        gt[:, (t % GATHER_HTILES) * BPH + ci, :],
                        start=True,
                        stop=True,
                    )
                hsb = hpool.tile([128, 512], dt.bfloat16)
                nc.scalar.activation(
                    hsb[:], hps[:], AF.Prelu, bias=0.0, scale=1.0, alpha=0.01
                )

                j = t % 3
                if j == 0:
                    otile = opsum.tile([96, 512], dt.float32, tag="o")
                for v in range(4):
                    nc.tensor.matmul(
                        otile[32 * j : 32 * j + 32, 128 * v : 128 * v + 128],
                        w2t[:, 32 * v : 32 * v + 32],
                        hsb[:, 128 * v : 128 * v + 128],
                        start=True,
                        stop=True,
                    )
                if j == 2:
                    osb = opool.tile([96, 512], dt.float32)
                    nc.vector.tensor_copy(osb[:], otile[:])
                    for jj in range(3):
                        th = t - 2 + jj
                        src = osb[32 * jj : 32 * jj + 24, :].rearrange(
                            "r (v b) -> r v b", v=4
                        )
                        dst = out[
                            GENES_PER_HTILE * th : GENES_PER_HTILE * (th + 1), :
                        ].rearrange("(v r) b -> r v b", v=4)
                        nc.sync.dma_start(dst, src)
    nc.compile()
    return nc


def _get_program():
    global _PROGRAM
    if _PROGRAM is None:
        _PROGRAM = _build_program()
    return _PROGRAM


def _prep_inputs(features, w1, b1, w2, b2, rows1, cols1, rows2, cols2):
    """Host-side staging: shard genes over cores, densify per-block weights.

    Returns in_maps: list (per core) of dicts with keys f/idx/w1b/w2b.
    """
    features = np.asarray(features, dtype=np.float32)
    w1 = np.asarray(w1, dtype=np.float32)
    b1 = np.asarray(b1, dtype=np.float32)
    w2 = np.asarray(w2, dtype=np.float32)
    b2 = np.asarray(b2, dtype=np.float32)
    cols1 = np.asarray(cols1)

    # Verify the structural assumption this kernel exploits.
    c1 = cols1.reshape(NUM_GENES, WIDTH, TPG)
    assert (c1 == c1[:, :1, :]).all(), "layer-1 TFs must be shared across WIDTH"
    tf_idx = c1[:, 0, :].astype(np.int64)  # [G, 10]

    w1g = w1.reshape(NUM_GENES, WIDTH, TPG)  # [G, 4, 10]
    b1g = b1.reshape(NUM_GENES, WIDTH)  # [G, 4]
    w2g = w2.reshape(NUM_GENES, WIDTH)  # [G, 4]

    # features.T with an appended ones row (bias carrier), bf16.
    F = np.empty((NUM_TFS + 1, BATCH), dtype=ml_dtypes.bfloat16)
    F[:NUM_TFS] = features.T.astype(ml_dtypes.bfloat16)
    F[NUM_TFS] = 1.0

    in_maps = []
    for c in range(NCORES):
        w1b = np.zeros((HTILES, 128, BPH * 64), dtype=np.float32)
        w2b = np.zeros((HTILES, 128, 4 * 32), dtype=np.float32)
        idxs = np.full((HTILES, BPH, 128), ONES_ROW, dtype=np.int64)
        for t in range(HTILES):
            for ci in range(BPH):
                u, v = ci // 4, ci % 4
                gs = 96 * t + 24 * v + 12 * u + np.arange(GPB)
                val = gs < GENES_PER_CORE
                g = np.where(val, c * GENES_PER_CORE + gs, 0)
                wv = w1g[g] * val[:, None, None]  # [12, 4, 10]
                for j in range(GPB):
                    # layer-1 weights: row 10j+tt, col 64ci + 4j+k
                    w1b[t, 10 * j : 10 * j + 10, 64 * ci + 4 * j : 64 * ci + 4 * j + 4] = (
                        wv[j].T
                    )
                    # b1 via the ones gather slot (row 120)
                    w1b[t, 120, 64 * ci + 4 * j : 64 * ci + 4 * j + 4] = (
                        b1g[g[j]] * val[j]
                    )
                    # layer-2 weights: row 64u+4j+k, col 24v + 12u + j
                    w2b[t, 64 * u + 4 * j : 64 * u + 4 * j + 4, 32 * v + 12 * u + j] = (
                        w2g[g[j]] * val[j]
                    )
                    # b2 via the ones h-row (48)
                    w2b[t, 48, 32 * v + 12 * u + j] = b2[g[j]] * val[j]
                if u == 0:
                    # h row 48 := 1.0 (written by the u=0 matmul's col 48)
                    w1b[t, 120, 64 * ci + 48] = 1.0
                idxs[t, ci, :120] = np.where(
                    val[:, None], tf_idx[g], ONES_ROW
                ).reshape(120)
        flat = idxs.reshape(NIDX).astype(np.int16)
        wrapped = np.tile(flat.reshape(NIDX // 16, 16).T, (8, 1))  # [128, NIDX/16]

        in_maps.append(
            {
                "f": F,
                "idx": np.ascontiguousarray(wrapped),
                "w1b": w1b.astype(ml_dtypes.bfloat16),
                "w2b": w2b.astype(ml_dtypes.bfloat16),
            }
        )
    return in_maps


def _assemble_output(core_outs):
    parts = [o[:GENES_PER_CORE] for o in core_outs]
    full = np.concatenate(parts, axis=0)  # [20000, 128]
    return np.ascontiguousarray(full.T.astype(np.float32))


def kernel(**inputs) -> np.ndarray:
    from concourse.bass_utils import run_bass_kernel_spmd

    nc = _get_program()
    in_maps = _prep_inputs(**inputs)
    res = run_bass_kernel_spmd(nc, in_maps, core_ids=list(range(NCORES)))
    return _assemble_output([r["out"] for r in res.results])


# revision 10
# speedup vs baseline: 2.0109x; 1.0301x over previous
"""Trainium2 Bass kernel for nn_AEDecoder (sparse 2-layer gene decoder).

Computation (reference):
    h   = leaky_relu(features @ W1s.T + b1, 0.01)   # W1s sparse [80000, 1600], 10 nnz/row
    out = h @ W2s.T + b2                             # gene g sums its 4 hidden nodes

Structure exploited (deterministic from the reference's connectivity builder):
  - hidden node h = 4g+k uses the *same* 10 TFs for all k of gene g
  - layer 2 is a grouped reduction over contiguous blocks of 4

Strategy: shard genes across the 8 cores (2500/core, full batch per core; no
cross-core comms). Per core: dma_gather rows of transposed bf16 features
(batch on the free axis, 256 B per gathered row), then per 12-gene block one
bf16 matmul [128 slots, 64]^T @ [128 slots, 128 batch] -> PSUM, one ScalarE
Lrelu pass per 8-block htile, then one layer-2 matmul per 128-batch slot
(2 blocks fused via zero-padded rows), biases folded into the matmuls via a
constant ones row gathered from a 1601st feature row.
"""

import sys

sys.path.insert(0, "/opt/trn_rl_repo")

import numpy as np
import ml_dtypes

# ---- problem constants (hardcoded; kernel.py must be self-contained) ----
NUM_TFS = 1600
NUM_GENES = 20000
WIDTH = 4
TPG = 10  # TFs per gene
HIDDEN = NUM_GENES * WIDTH
BATCH = 128
NCORES = 8
GENES_PER_CORE = NUM_GENES // NCORES  # 2500

# ---- block geometry ----
GPB = 12  # genes per block (120 slots + 8 pad = 128 partitions)
BPH = 8  # blocks per htile: 2 col groups (u) x 4 free slots (v)
GENES_PER_HTILE = GPB * BPH  # 96
HTILES = -(-GENES_PER_CORE // GENES_PER_HTILE)  # 27
GENES_PAD = HTILES * GENES_PER_HTILE  # 2592
BLOCKS = HTILES * BPH  # 216
NIDX = BLOCKS * 128  # 27648 gather slots per core
GATHER_HTILES = 1  # htiles per dma_gather (HW limit: <=1024 idxs per gather)
ONES_ROW = NUM_TFS  # feature row index holding constant 1.0

_PROGRAM = None


def _build_program(htiles=HTILES):
    import concourse.bacc as bacc
    import concourse.mybir as mybir
    from concourse import tile
    from concourse.library_config import mlp

    dt = mybir.dt
    AF = mybir.ActivationFunctionType

    nc = bacc.Bacc("TRN2", num_swdge_queues=4)
    f = nc.dram_tensor("f", [NUM_TFS + 1, BATCH], dt.bfloat16, kind="ExternalInput")
    idx = nc.dram_tensor("idx", [128, NIDX // 16], dt.int16, kind="ExternalInput")
    w1 = nc.dram_tensor(
        "w1b", [HTILES, 128, BPH * 64], dt.bfloat16, kind="ExternalInput"
    )
    w2 = nc.dram_tensor("w2b", [HTILES, 128, 4 * 32], dt.bfloat16, kind="ExternalInput")
    out = nc.dram_tensor("out", [GENES_PAD, BATCH], dt.float32, kind="ExternalOutput")

    WCHUNK = 7  # htiles per weight-preload DMA
    with tile.TileContext(nc) as tc:
        nc.gpsimd.load_library(mlp)
        with (
            tc.tile_pool(name="idxp", bufs=1) as idxpool,
            tc.tile_pool(name="g", bufs=6) as gpool,
            tc.tile_pool(name="w1p", bufs=1) as w1pool,
            tc.tile_pool(name="w2p", bufs=1) as w2pool,
            tc.tile_pool(name="h", bufs=4) as hpool,
            tc.tile_pool(name="o", bufs=2) as opool,
            tc.tile_pool(name="hps", bufs=3, space="PSUM") as hpsum,
            tc.tile_pool(name="ops", bufs=2, space="PSUM") as opsum,
        ):
            idxt = idxpool.tile([128, NIDX // 16], dt.int16)
            nc.gpsimd.dma_start(idxt[:], idx[:])
            # preload all block weights up front (keeps the Sync HWDGE ring
            # free of out-DMA completions that would stall late ldweights)
            w1t = w1pool.tile([128, HTILES * BPH * 64], dt.bfloat16)
            w2t = w2pool.tile([128, HTILES * 4 * 32], dt.bfloat16)
            for t0 in range(0, HTILES, WCHUNK):
                t1 = min(t0 + WCHUNK, HTILES)
                nt = t1 - t0
                nc.sync.dma_start(
                    w1t[:, t0 * 512 : t1 * 512].rearrange("p (t c) -> p t c", t=nt),
                    w1[t0:t1].rearrange("t p c -> p t c"),
                )
                nc.sync.dma_start(
                    w2t[:, t0 * 128 : t1 * 128].rearrange("p (t c) -> p t c", t=nt),
                    w2[t0:t1].rearrange("t p c -> p t c"),
                )

            gt = None
            otile = None
            for t in range(htiles):
                if t % GATHER_HTILES == 0:
                    nh = min(GATHER_HTILES, htiles - t)
                    n_i = nh * BPH * 128
                    gt = gpool.tile([128, nh * BPH, 128], dt.bfloat16, tag="g")
                    c0 = t * BPH * 128 // 16
                    nc.gpsimd.dma_gather(
                        gt[:],
                        f[:],
                        idxt[:, c0 : c0 + n_i // 16],
                        n_i,
                        n_i,
                        BATCH,
                        queue_num=t % 4,
                    )
                hps = hpsum.tile([128, 512], dt.float32)
                for ci in range(BPH):
                    u, v = ci // 4, ci % 4
                    nc.tensor.matmul(
                        hps[64 * u : 64 * u + 64, 128 * v : 128 * v + 128],
                        w1t[:, t * 512 + 64 * ci : t * 512 + 64 * ci + 64],
                        gt[:, (t % GATHER_HTILES) * BPH + ci, :],
                        start=True,
                        stop=True,
                    )
                hsb = hpool.tile([128, 512], dt.bfloat16)
                nc.scalar.activation(
                    hsb[:], hps[:], AF.Prelu, bias=0.0, scale=1.0, alpha=0.01
                )

                j = t % 3
                if j == 0:
                    otile = opsum.tile([96, 512], dt.float32, tag="o")
                for v in range(4):
                    nc.tensor.matmul(
                        otile[32 * j : 32 * j + 32, 128 * v : 128 * v + 128],
                        w2t[:, t * 128 + 32 * v : t * 128 + 32 * v + 32],
                        hsb[:, 128 * v : 128 * v + 128],
                        start=True,
                        stop=True,
                    )
                if j == 2:
                    osb = opool.tile([96, 512], dt.float32)
                    nc.vector.tensor_copy(osb[:], otile[:])
                    for jj in range(3):
                        th = t - 2 + jj
                        src = osb[32 * jj : 32 * jj + 24, :].rearrange(
                            "r (v b) -> r v b", v=4
                        )
                        dst = out[
                            GENES_PER_HTILE * th : GENES_PER_HTILE * (th + 1), :
                        ].rearrange("(v r) b -> r v b", v=4)
                        nc.scalar.dma_start(dst, src)
    nc.compile()
    return nc


def _get_program():
    global _PROGRAM
    if _PROGRAM is None:
        _PROGRAM = _build_program()
    return _PROGRAM


def _prep_inputs(features, w1, b1, w2, b2, rows1, cols1, rows2, cols2):
    """Host-side staging: shard genes over cores, densify per-block weights.

    Returns in_maps: list (per core) of dicts with keys f/idx/w1b/w2b.
    """
    features = np.asarray(features, dtype=np.float32)
    w1 = np.asarray(w1, dtype=np.float32)
    b1 = np.asarray(b1, dtype=np.float32)
    w2 = np.asarray(w2, dtype=np.float32)
    b2 = np.asarray(b2, dtype=np.float32)
    cols1 = np.asarray(cols1)

    # Verify the structural assumption this kernel exploits.
    c1 = cols1.reshape(NUM_GENES, WIDTH, TPG)
    assert (c1 == c1[:, :1, :]).all(), "layer-1 TFs must be shared across WIDTH"
    tf_idx = c1[:, 0, :].astype(np.int64)  # [G, 10]

    w1g = w1.reshape(NUM_GENES, WIDTH, TPG)  # [G, 4, 10]
    b1g = b1.reshape(NUM_GENES, WIDTH)  # [G, 4]
    w2g = w2.reshape(NUM_GENES, WIDTH)  # [G, 4]

    # features.T with an appended ones row (bias carrier), bf16.
    F = np.empty((NUM_TFS + 1, BATCH), dtype=ml_dtypes.bfloat16)
    F[:NUM_TFS] = features.T.astype(ml_dtypes.bfloat16)
    F[NUM_TFS] = 1.0

    in_maps = []
    for c in range(NCORES):
        w1b = np.zeros((HTILES, 128, BPH * 64), dtype=np.float32)
        w2b = np.zeros((HTILES, 128, 4 * 32), dtype=np.float32)
        idxs = np.full((HTILES, BPH, 128), ONES_ROW, dtype=np.int64)
        for t in range(HTILES):
            for ci in range(BPH):
                u, v = ci // 4, ci % 4
                gs = 96 * t + 24 * v + 12 * u + np.arange(GPB)
                val = gs < GENES_PER_CORE
                g = np.where(val, c * GENES_PER_CORE + gs, 0)
                wv = w1g[g] * val[:, None, None]  # [12, 4, 10]
                for j in range(GPB):
                    # layer-1 weights: row 10j+tt, col 64ci + 4j+k
                    w1b[t, 10 * j : 10 * j + 10, 64 * ci + 4 * j : 64 * ci + 4 * j + 4] = (
                        wv[j].T
                    )
                    # b1 via the ones gather slot (row 120)
                    w1b[t, 120, 64 * ci + 4 * j : 64 * ci + 4 * j + 4] = (
                        b1g[g[j]] * val[j]
                    )
                    # layer-2 weights: row 64u+4j+k, col 24v + 12u + j
                    w2b[t, 64 * u + 4 * j : 64 * u + 4 * j + 4, 32 * v + 12 * u + j] = (
                        w2g[g[j]] * val[j]
                    )
                    # b2 via the ones h-row (48)
                    w2b[t, 48, 32 * v + 12 * u + j] = b2[g[j]] * val[j]
                if u == 0:
                    # h row 48 := 1.0 (written by the u=0 matmul's col 48)
                    w1b[t, 120, 64 * ci + 48] = 1.0
                idxs[t, ci, :120] = np.where(
                    val[:, None], tf_idx[g], ONES_ROW
                ).reshape(120)
        flat = idxs.reshape(NIDX).astype(np.int16)
        wrapped = np.tile(flat.reshape(NIDX // 16, 16).T, (8, 1))  # [128, NIDX/16]

        in_maps.append(
            {
                "f": F,
                "idx": np.ascontiguousarray(wrapped),
                "w1b": w1b.astype(ml_dtypes.bfloat16),
                "w2b": w2b.astype(ml_dtypes.bfloat16),
            }
        )
    return in_maps


def _assemble_output(core_outs):
    parts = [o[:GENES_PER_CORE] for o in core_outs]
    full = np.concatenate(parts, axis=0)  # [20000, 128]
    return np.ascontiguousarray(full.T.astype(np.float32))


def kernel(**inputs) -> np.ndarray:
    from concourse.bass_utils import run_bass_kernel_spmd

    nc = _get_program()
    in_maps = _prep_inputs(**inputs)
    res = run_bass_kernel_spmd(nc, in_maps, core_ids=list(range(NCORES)))
    return _assemble_output([r["out"] for r in res.results])


# revision 11
# speedup vs baseline: 2.2775x; 1.1326x over previous
"""Trainium2 Bass kernel for nn_AEDecoder (sparse 2-layer gene decoder).

Computation (reference):
    h   = leaky_relu(features @ W1s.T + b1, 0.01)   # W1s sparse [80000, 1600], 10 nnz/row
    out = h @ W2s.T + b2                             # gene g sums its 4 hidden nodes

Structure exploited (deterministic from the reference's connectivity builder):
  - hidden node h = 4g+k uses the *same* 10 TFs for all k of gene g
  - layer 2 is a grouped reduction over contiguous blocks of 4

Strategy: shard genes across the 8 cores (2500/core, full batch per core; no
cross-core comms). Per core: dma_gather rows of transposed bf16 features
(batch on the free axis, 256 B per gathered row), then per 12-gene block one
bf16 matmul [128 slots, 64]^T @ [128 slots, 128 batch] -> PSUM, one ScalarE
Lrelu pass per 8-block htile, then one layer-2 matmul per 128-batch slot
(2 blocks fused via zero-padded rows), biases folded into the matmuls via a
constant ones row gathered from a 1601st feature row.
"""

import sys

sys.path.insert(0, "/opt/trn_rl_repo")

import numpy as np
import ml_dtypes

# ---- problem constants (hardcoded; kernel.py must be self-contained) ----
NUM_TFS = 1600
NUM_GENES = 20000
WIDTH = 4
TPG = 10  # TFs per gene
HIDDEN = NUM_GENES * WIDTH
BATCH = 128
NCORES = 8
GENES_PER_CORE = NUM_GENES // NCORES  # 2500

# ---- block geometry ----
GPB = 12  # genes per block (120 slots + 8 pad = 128 partitions)
BPH = 8  # blocks per htile: 2 col groups (u) x 4 free slots (v)
GENES_PER_HTILE = GPB * BPH  # 96
HTILES = -(-GENES_PER_CORE // GENES_PER_HTILE)  # 27
GENES_PAD = HTILES * GENES_PER_HTILE  # 2592
BLOCKS = HTILES * BPH  # 216
NIDX = BLOCKS * 128  # 27648 gather slots per core
GATHER_HTILES = 1  # htiles per dma_gather (HW limit: <=1024 idxs per gather)
ONES_ROW = NUM_TFS  # feature row index holding constant 1.0

_PROGRAM = None


def _build_program(htiles=HTILES):
    import concourse.bacc as bacc
    import concourse.mybir as mybir
    from concourse import tile
    from concourse.library_config import mlp

    dt = mybir.dt
    AF = mybir.ActivationFunctionType

    nc = bacc.Bacc("TRN2", num_swdge_queues=4)
    f = nc.dram_tensor("f", [NUM_TFS + 1, BATCH], dt.bfloat16, kind="ExternalInput")
    idx = nc.dram_tensor("idx", [128, NIDX // 16], dt.int16, kind="ExternalInput")
    w1 = nc.dram_tensor(
        "w1b", [HTILES, 128, BPH * 64], dt.bfloat16, kind="ExternalInput"
    )
    w2 = nc.dram_tensor("w2b", [HTILES, 128, 4 * 32], dt.bfloat16, kind="ExternalInput")
    out = nc.dram_tensor("out", [GENES_PAD, BATCH], dt.float32, kind="ExternalOutput")

    WCHUNK = 7  # htiles per weight-preload DMA
    with tile.TileContext(nc) as tc:
        nc.gpsimd.load_library(mlp)
        with (
            tc.tile_pool(name="idxp", bufs=1) as idxpool,
            tc.tile_pool(name="g", bufs=6) as gpool,
            tc.tile_pool(name="w1p", bufs=1) as w1pool,
            tc.tile_pool(name="w2p", bufs=1) as w2pool,
            tc.tile_pool(name="h", bufs=4) as hpool,
            tc.tile_pool(name="o", bufs=4) as opool,
            tc.tile_pool(name="hps", bufs=4, space="PSUM") as hpsum,
            tc.tile_pool(name="ops", bufs=3, space="PSUM") as opsum,
        ):
            idxt = idxpool.tile([128, NIDX // 16], dt.int16)
            nc.sync.dma_start(idxt[:], idx[:])
            # preload all block weights up front (keeps the Sync HWDGE ring
            # free of out-DMA completions that would stall late ldweights)
            w1t = w1pool.tile([128, HTILES * BPH * 64], dt.bfloat16)
            w2t = w2pool.tile([128, HTILES * 4 * 32], dt.bfloat16)
            for t0 in range(0, HTILES, WCHUNK):
                t1 = min(t0 + WCHUNK, HTILES)
                nt = t1 - t0
                nc.sync.dma_start(
                    w1t[:, t0 * 512 : t1 * 512].rearrange("p (t c) -> p t c", t=nt),
                    w1[t0:t1].rearrange("t p c -> p t c"),
                )
                nc.sync.dma_start(
                    w2t[:, t0 * 128 : t1 * 128].rearrange("p (t c) -> p t c", t=nt),
                    w2[t0:t1].rearrange("t p c -> p t c"),
                )

            gt = None
            otile = None
            for t in range(htiles):
                if t % GATHER_HTILES == 0:
                    nh = min(GATHER_HTILES, htiles - t)
                    n_i = nh * BPH * 128
                    gt = gpool.tile([128, nh * BPH, 128], dt.bfloat16, tag="g")
                    c0 = t * BPH * 128 // 16
                    nc.gpsimd.dma_gather(
                        gt[:],
                        f[:],
                        idxt[:, c0 : c0 + n_i // 16],
                        n_i,
                        n_i,
                        BATCH,
                        queue_num=t % 4,
                    )
                hps = hpsum.tile([128, 512], dt.float32)
                for ci in range(BPH):
                    u, v = ci // 4, ci % 4
                    nc.tensor.matmul(
                        hps[64 * u : 64 * u + 64, 128 * v : 128 * v + 128],
                        w1t[:, t * 512 + 64 * ci : t * 512 + 64 * ci + 64],
                        gt[:, (t % GATHER_HTILES) * BPH + ci, :],
                        start=True,
                        stop=True,
                    )
                hsb = hpool.tile([128, 512], dt.bfloat16)
                nc.scalar.activation(
                    hsb[:], hps[:], AF.Prelu, bias=0.0, scale=1.0, alpha=0.01
                )

                j = t % 3
                if j == 0:
                    otile = opsum.tile([96, 512], dt.float32, tag="o")
                for v in range(4):
                    nc.tensor.matmul(
                        otile[32 * j : 32 * j + 32, 128 * v : 128 * v + 128],
                        w2t[:, t * 128 + 32 * v : t * 128 + 32 * v + 32],
                        hsb[:, 128 * v : 128 * v + 128],
                        start=True,
                        stop=True,
                    )
                if j == 2:
                    osb = opool.tile([96, 512], dt.float32)
                    nc.vector.tensor_copy(osb[:], otile[:])
                    for jj in range(3):
                        th = t - 2 + jj
                        src = osb[32 * jj : 32 * jj + 24, :].rearrange(
                            "r (v b) -> r v b", v=4
                        )
                        dst = out[
                            GENES_PER_HTILE * th : GENES_PER_HTILE * (th + 1), :
                        ].rearrange("(v r) b -> r v b", v=4)
                        nc.scalar.dma_start(dst, src)
    nc.compile()
    return nc


def _get_program():
    global _PROGRAM
    if _PROGRAM is None:
        _PROGRAM = _build_program()
    return _PROGRAM


def _prep_inputs(features, w1, b1, w2, b2, rows1, cols1, rows2, cols2):
    """Host-side staging: shard genes over cores, densify per-block weights.

    Returns in_maps: list (per core) of dicts with keys f/idx/w1b/w2b.
    """
    features = np.asarray(features, dtype=np.float32)
    w1 = np.asarray(w1, dtype=np.float32)
    b1 = np.asarray(b1, dtype=np.float32)
    w2 = np.asarray(w2, dtype=np.float32)
    b2 = np.asarray(b2, dtype=np.float32)
    cols1 = np.asarray(cols1)

    # Verify the structural assumption this kernel exploits.
    c1 = cols1.reshape(NUM_GENES, WIDTH, TPG)
    assert (c1 == c1[:, :1, :]).all(), "layer-1 TFs must be shared across WIDTH"
    tf_idx = c1[:, 0, :].astype(np.int64)  # [G, 10]

    w1g = w1.reshape(NUM_GENES, WIDTH, TPG)  # [G, 4, 10]
    b1g = b1.reshape(NUM_GENES, WIDTH)  # [G, 4]
    w2g = w2.reshape(NUM_GENES, WIDTH)  # [G, 4]

    # features.T with an appended ones row (bias carrier), bf16.
    F = np.empty((NUM_TFS + 1, BATCH), dtype=ml_dtypes.bfloat16)
    F[:NUM_TFS] = features.T.astype(ml_dtypes.bfloat16)
    F[NUM_TFS] = 1.0

    in_maps = []
    for c in range(NCORES):
        w1b = np.zeros((HTILES, 128, BPH * 64), dtype=np.float32)
        w2b = np.zeros((HTILES, 128, 4 * 32), dtype=np.float32)
        idxs = np.full((HTILES, BPH, 128), ONES_ROW, dtype=np.int64)
        for t in range(HTILES):
            for ci in range(BPH):
                u, v = ci // 4, ci % 4
                gs = 96 * t + 24 * v + 12 * u + np.arange(GPB)
                val = gs < GENES_PER_CORE
                g = np.where(val, c * GENES_PER_CORE + gs, 0)
                wv = w1g[g] * val[:, None, None]  # [12, 4, 10]
                for j in range(GPB):
                    # layer-1 weights: row 10j+tt, col 64ci + 4j+k
                    w1b[t, 10 * j : 10 * j + 10, 64 * ci + 4 * j : 64 * ci + 4 * j + 4] = (
                        wv[j].T
                    )
                    # b1 via the ones gather slot (row 120)
                    w1b[t, 120, 64 * ci + 4 * j : 64 * ci + 4 * j + 4] = (
                        b1g[g[j]] * val[j]
                    )
                    # layer-2 weights: row 64u+4j+k, col 24v + 12u + j
                    w2b[t, 64 * u + 4 * j : 64 * u + 4 * j + 4, 32 * v + 12 * u + j] = (
                        w2g[g[j]] * val[j]
                    )
                    # b2 via the ones h-row (48)
                    w2b[t, 48, 32 * v + 12 * u + j] = b2[g[j]] * val[j]
                if u == 0:
                    # h row 48 := 1.0 (written by the u=0 matmul's col 48)
                    w1b[t, 120, 64 * ci + 48] = 1.0
                idxs[t, ci, :120] = np.where(
                    val[:, None], tf_idx[g], ONES_ROW
                ).reshape(120)
        flat = idxs.reshape(NIDX).astype(np.int16)
        wrapped = np.tile(flat.reshape(NIDX // 16, 16).T, (8, 1))  # [128, NIDX/16]

        in_maps.append(
            {
                "f": F,
                "idx": np.ascontiguousarray(wrapped),
                "w1b": w1b.astype(ml_dtypes.bfloat16),
                "w2b": w2b.astype(ml_dtypes.bfloat16),
            }
        )
    return in_maps


def _assemble_output(core_outs):
    parts = [o[:GENES_PER_CORE] for o in core_outs]
    full = np.concatenate(parts, axis=0)  # [20000, 128]
    return np.ascontiguousarray(full.T.astype(np.float32))


def kernel(**inputs) -> np.ndarray:
    from concourse.bass_utils import run_bass_kernel_spmd

    nc = _get_program()
    in_maps = _prep_inputs(**inputs)
    res = run_bass_kernel_spmd(nc, in_maps, core_ids=list(range(NCORES)))
    return _assemble_output([r["out"] for r in res.results])


# revision 12
# speedup vs baseline: 2.4417x; 1.0721x over previous
"""Trainium2 Bass kernel for nn_AEDecoder (sparse 2-layer gene decoder).

Computation (reference):
    h   = leaky_relu(features @ W1s.T + b1, 0.01)   # W1s sparse [80000, 1600], 10 nnz/row
    out = h @ W2s.T + b2                             # gene g sums its 4 hidden nodes

Structure exploited (deterministic from the reference's connectivity builder):
  - hidden node h = 4g+k uses the *same* 10 TFs for all k of gene g
  - layer 2 is a grouped reduction over contiguous blocks of 4

Strategy: shard genes across the 8 cores (2500/core, full batch per core; no
cross-core comms). Per core: dma_gather rows of transposed bf16 features
(batch on the free axis, 256 B per gathered row), then per 12-gene block one
bf16 matmul [128 slots, 64]^T @ [128 slots, 128 batch] -> PSUM, one ScalarE
Lrelu pass per 8-block htile, then one layer-2 matmul per 128-batch slot
(2 blocks fused via zero-padded rows), biases folded into the matmuls via a
constant ones row gathered from a 1601st feature row.
"""

import sys

sys.path.insert(0, "/opt/trn_rl_repo")

import numpy as np
import ml_dtypes

# ---- problem constants (hardcoded; kernel.py must be self-contained) ----
NUM_TFS = 1600
NUM_GENES = 20000
WIDTH = 4
TPG = 10  # TFs per gene
HIDDEN = NUM_GENES * WIDTH
BATCH = 128
NCORES = 8
GENES_PER_CORE = NUM_GENES // NCORES  # 2500

# ---- block geometry ----
GPB = 12  # genes per block (120 slots + 8 pad = 128 partitions)
BPH = 8  # blocks per htile: 2 col groups (u) x 4 free slots (v)
GENES_PER_HTILE = GPB * BPH  # 96
HTILES = -(-GENES_PER_CORE // GENES_PER_HTILE)  # 27
GENES_PAD = HTILES * GENES_PER_HTILE  # 2592
BLOCKS = HTILES * BPH  # 216
NIDX = BLOCKS * 128  # 27648 gather slots per core
GATHER_HTILES = 1  # htiles per dma_gather (HW limit: <=1024 idxs per gather)
ONES_ROW = NUM_TFS  # feature row index holding constant 1.0

_PROGRAM = None


def _build_program(htiles=HTILES):
    import concourse.bacc as bacc
    import concourse.mybir as mybir
    from concourse import tile
    from concourse.library_config import mlp

    dt = mybir.dt
    AF = mybir.ActivationFunctionType

    nc = bacc.Bacc("TRN2", num_swdge_queues=4)
    f = nc.dram_tensor("f", [NUM_TFS + 1, BATCH], dt.bfloat16, kind="ExternalInput")
    idx = nc.dram_tensor("idx", [128, NIDX // 16], dt.int16, kind="ExternalInput")
    w1 = nc.dram_tensor(
        "w1b", [HTILES, 128, BPH * 64], dt.bfloat16, kind="ExternalInput"
    )
    w2 = nc.dram_tensor("w2b", [HTILES, 128, 4 * 32], dt.bfloat16, kind="ExternalInput")
    out = nc.dram_tensor("out", [GENES_PAD, BATCH], dt.float32, kind="ExternalOutput")

    WCHUNK = 7  # htiles per weight-preload DMA
    with tile.TileContext(nc) as tc:
        nc.gpsimd.load_library(mlp)
        with (
            tc.tile_pool(name="idxp", bufs=1) as idxpool,
            tc.tile_pool(name="g", bufs=6) as gpool,
            tc.tile_pool(name="w1p", bufs=1) as w1pool,
            tc.tile_pool(name="w2p", bufs=1) as w2pool,
            tc.tile_pool(name="h", bufs=4) as hpool,
            tc.tile_pool(name="o", bufs=4) as opool,
            tc.tile_pool(name="hps", bufs=4, space="PSUM") as hpsum,
            tc.tile_pool(name="ops", bufs=3, space="PSUM") as opsum,
        ):
            idxt = idxpool.tile([128, NIDX // 16], dt.int16)
            nc.sync.dma_start(idxt[:], idx[:])
            # preload all block weights up front (keeps the Sync HWDGE ring
            # free of out-DMA completions that would stall late ldweights)
            w1t = w1pool.tile([128, HTILES * BPH * 64], dt.bfloat16)
            w2t = w2pool.tile([128, HTILES * 4 * 32], dt.bfloat16)
            for t0 in range(0, HTILES, WCHUNK):
                t1 = min(t0 + WCHUNK, HTILES)
                nt = t1 - t0
                nc.sync.dma_start(
                    w1t[:, t0 * 512 : t1 * 512].rearrange("p (t c) -> p t c", t=nt),
                    w1[t0:t1].rearrange("t p c -> p t c"),
                )
                nc.sync.dma_start(
                    w2t[:, t0 * 128 : t1 * 128].rearrange("p (t c) -> p t c", t=nt),
                    w2[t0:t1].rearrange("t p c -> p t c"),
                )

            gt = None
            otile = None
            for t in range(htiles):
                if t % GATHER_HTILES == 0:
                    nh = min(GATHER_HTILES, htiles - t)
                    n_i = nh * BPH * 128
                    gt = gpool.tile([128, nh * BPH, 128], dt.bfloat16, tag="g")
                    c0 = t * BPH * 128 // 16
                    nc.gpsimd.dma_gather(
                        gt[:],
                        f[:],
                        idxt[:, c0 : c0 + n_i // 16],
                        n_i,
                        n_i,
                        BATCH,
                        queue_num=t % 4,
                    )
                hps = hpsum.tile([128, 512], dt.float32)
                for ci in range(BPH):
                    u, v = ci // 4, ci % 4
                    nc.tensor.matmul(
                        hps[64 * u : 64 * u + 64, 128 * v : 128 * v + 128],
                        w1t[:, t * 512 + 64 * ci : t * 512 + 64 * ci + 64],
                        gt[:, (t % GATHER_HTILES) * BPH + ci, :],
                        start=True,
                        stop=True,
                    )
                hsb = hpool.tile([128, 512], dt.bfloat16)
                nc.scalar.activation(
                    hsb[:], hps[:], AF.Prelu, bias=0.0, scale=1.0, alpha=0.01
                )

                j = t % 3
                if j == 0:
                    otile = opsum.tile([96, 512], dt.float32, tag="o")
                for v in range(4):
                    nc.tensor.matmul(
                        otile[32 * j : 32 * j + 32, 128 * v : 128 * v + 128],
                        w2t[:, t * 128 + 32 * v : t * 128 + 32 * v + 32],
                        hsb[:, 128 * v : 128 * v + 128],
                        start=True,
                        stop=True,
                    )
                if j == 2:
                    osb = opool.tile([96, 512], dt.float32)
                    nc.vector.tensor_copy(osb[:], otile[:])
                    for jj in range(3):
                        th = t - 2 + jj
                        src = osb[32 * jj : 32 * jj + 24, :].rearrange(
                            "r (v b) -> r v b", v=4
                        )
                        dst = out[
                            GENES_PER_HTILE * th : GENES_PER_HTILE * (th + 1), :
                        ].rearrange("(v r) b -> r v b", v=4)
                        nc.sync.dma_start(dst, src)
    nc.compile()
    return nc


def _get_program():
    global _PROGRAM
    if _PROGRAM is None:
        _PROGRAM = _build_program()
    return _PROGRAM


def _prep_inputs(features, w1, b1, w2, b2, rows1, cols1, rows2, cols2):
    """Host-side staging: shard genes over cores, densify per-block weights.

    Returns in_maps: list (per core) of dicts with keys f/idx/w1b/w2b.
    """
    features = np.asarray(features, dtype=np.float32)
    w1 = np.asarray(w1, dtype=np.float32)
    b1 = np.asarray(b1, dtype=np.float32)
    w2 = np.asarray(w2, dtype=np.float32)
    b2 = np.asarray(b2, dtype=np.float32)
    cols1 = np.asarray(cols1)

    # Verify the structural assumption this kernel exploits.
    c1 = cols1.reshape(NUM_GENES, WIDTH, TPG)
    assert (c1 == c1[:, :1, :]).all(), "layer-1 TFs must be shared across WIDTH"
    tf_idx = c1[:, 0, :].astype(np.int64)  # [G, 10]

    w1g = w1.reshape(NUM_GENES, WIDTH, TPG)  # [G, 4, 10]
    b1g = b1.reshape(NUM_GENES, WIDTH)  # [G, 4]
    w2g = w2.reshape(NUM_GENES, WIDTH)  # [G, 4]

    # features.T with an appended ones row (bias carrier), bf16.
    F = np.empty((NUM_TFS + 1, BATCH), dtype=ml_dtypes.bfloat16)
    F[:NUM_TFS] = features.T.astype(ml_dtypes.bfloat16)
    F[NUM_TFS] = 1.0

    in_maps = []
    for c in range(NCORES):
        w1b = np.zeros((HTILES, 128, BPH * 64), dtype=np.float32)
        w2b = np.zeros((HTILES, 128, 4 * 32), dtype=np.float32)
        idxs = np.full((HTILES, BPH, 128), ONES_ROW, dtype=np.int64)
        for t in range(HTILES):
            for ci in range(BPH):
                u, v = ci // 4, ci % 4
                gs = 96 * t + 24 * v + 12 * u + np.arange(GPB)
                val = gs < GENES_PER_CORE
                g = np.where(val, c * GENES_PER_CORE + gs, 0)
                wv = w1g[g] * val[:, None, None]  # [12, 4, 10]
                for j in range(GPB):
                    # layer-1 weights: row 10j+tt, col 64ci + 4j+k
                    w1b[t, 10 * j : 10 * j + 10, 64 * ci + 4 * j : 64 * ci + 4 * j + 4] = (
                        wv[j].T
                    )
                    # b1 via the ones gather slot (row 120)
                    w1b[t, 120, 64 * ci + 4 * j : 64 * ci + 4 * j + 4] = (
                        b1g[g[j]] * val[j]
                    )
                    # layer-2 weights: row 64u+4j+k, col 24v + 12u + j
                    w2b[t, 64 * u + 4 * j : 64 * u + 4 * j + 4, 32 * v + 12 * u + j] = (
                        w2g[g[j]] * val[j]
                    )
                    # b2 via the ones h-row (48)
                    w2b[t, 48, 32 * v + 12 * u + j] = b2[g[j]] * val[j]
                if u == 0:
                    # h row 48 := 1.0 (written by the u=0 matmul's col 48)
                    w1b[t, 120, 64 * ci + 48] = 1.0
                idxs[t, ci, :120] = np.where(
                    val[:, None], tf_idx[g], ONES_ROW
                ).reshape(120)
        flat = idxs.reshape(NIDX).astype(np.int16)
        wrapped = np.tile(flat.reshape(NIDX // 16, 16).T, (8, 1))  # [128, NIDX/16]

        in_maps.append(
            {
                "f": F,
                "idx": np.ascontiguousarray(wrapped),
                "w1b": w1b.astype(ml_dtypes.bfloat16),
                "w2b": w2b.astype(ml_dtypes.bfloat16),
            }
        )
    return in_maps


def _assemble_output(core_outs):
    parts = [o[:GENES_PER_CORE] for o in core_outs]
    full = np.concatenate(parts, axis=0)  # [20000, 128]
    return np.ascontiguousarray(full.T.astype(np.float32))


def kernel(**inputs) -> np.ndarray:
    from concourse.bass_utils import run_bass_kernel_spmd

    nc = _get_program()
    in_maps = _prep_inputs(**inputs)
    res = run_bass_kernel_spmd(nc, in_maps, core_ids=list(range(NCORES)))
    return _assemble_output([r["out"] for r in res.results])
